# revision 11
# baseline (speedup 1.0000x reference)
"""MixHop GNN (2-hop GCN propagation + MLP head) on 8 Trainium2 NeuronCores.

Strategy (node-sharded by dst, streaming — no on-device gather):
  norm factorization:  norm = dis[src]*dis[dst] ->  hop(v) = dis * S(dis * v)
  with S the plain scatter-sum over edges; self loops handled as a direct
  per-node add in the tail (never materialized as edges).

  Between launches the HOST materializes the per-edge value stream
  v_e = u[src_e] (fp8) in dst-grouped k-tile order, plus a STATIC fp8
  one-hot stream for the scatter matrices (built once, reused by both
  hops; dst tiles are 64 wide to halve the one-hot bytes).  Each core
  consumes both streams SEQUENTIALLY with fat DMA descriptors; the
  scatter-sum runs on the Tensor engine as
      psum[dst_tile 64, H] += OneHotT(fp8)^T @ v_tile(fp8)
  PSUM accumulation is in-order => no scatter races.  No SWDGE descriptor
  generation, no random-access DMA, no on-chip one-hot build.

  3 SPMD launches over 8 cores:
    L1: h = relu(x@w1+b1), u0 = dis*h                (row shard per core)
    L2: hop1 over u0-stream -> h1, u1 shards
    L3: hop2 over u1-stream + dense MLP tail -> log_softmax logits shard
"""

import numpy as np
import ml_dtypes

import concourse.bacc as bacc
import concourse.bass as bass
import concourse.tile as tile
from concourse import mybir
from concourse.bass_utils import run_bass_kernel_spmd

BF16 = ml_dtypes.bfloat16
FP8 = ml_dtypes.float8_e4m3
AF = mybir.ActivationFunctionType
ALU = mybir.AluOpType

N, E, F_IN, H, C = 100000, 1600000, 256, 64, 40
NCORE = 8
NSH = N // NCORE            # 12500 nodes per core
NT = (NSH + 127) // 128     # 98 128-row tiles per core (L1 + logits layout)
NTP = NT * 128              # 12544 padded rows
TW = 64                     # dst tile width for the hop scatter
NT2 = NTP // TW             # 196 hop dst tiles per core
VB = 96                     # k-tiles per stream staging block
TB = 4                      # 128-col blocks per dense-tail step in L3

_cache = {}
_last_runs = []


# --------------------------------------------------------------------------
# host-side graph partitioning / padding plan
# --------------------------------------------------------------------------

def _prep_graph(edge_index):
    src = np.asarray(edge_index[0], dtype=np.int64)
    dst = np.asarray(edge_index[1], dtype=np.int64)
    deg = (np.bincount(dst, minlength=N) + 1).astype(np.float32)  # + self loop
    dis = (1.0 / np.sqrt(deg)).astype(np.float32)

    per_core = []
    cnts = np.zeros((NCORE, NT2), np.int64)
    for c in range(NCORE):
        sel = (dst // NSH) == c
        s_g = src[sel]
        d_l = (dst[sel] - c * NSH).astype(np.int64)
        t_id = d_l // TW
        order = np.argsort(t_id, kind="stable")
        s_g, d_l, t_id = s_g[order], d_l[order], t_id[order]
        cnts[c] = np.bincount(t_id, minlength=NT2)
        per_core.append((s_g, d_l, t_id))

    nkt_t = np.ceil(cnts.max(axis=0) / 128.0).astype(np.int64)  # k-tiles/seg
    nkt_t = np.maximum(nkt_t, 1)
    off_t = np.zeros(NT2 + 1, np.int64)
    np.cumsum(nkt_t, out=off_t[1:])
    NKT = int(off_t[-1])

    srcs, ohs = [], []
    for c in range(NCORE):
        s_g, d_l, t_id = per_core[c]
        start = np.zeros(NT2, np.int64)
        np.cumsum(cnts[c], out=start)
        start = np.concatenate([[0], start[:-1]])
        rank = np.arange(len(t_id)) - start[t_id]
        pos = off_t[t_id] * 128 + rank
        stream_src = np.zeros(NKT * 128, np.int64)
        stream_src[pos] = s_g
        srcs.append(stream_src)
        # static fp8 one-hot stream: row e has 1 at dst_local_in_tile
        oh = np.zeros((NKT * 128, TW), FP8)
        oh[pos, d_l - TW * t_id] = 1
        ohs.append(np.ascontiguousarray(
            oh.reshape(NKT, 128, TW).transpose(1, 0, 2)
            .reshape(128, NKT * TW)))

    plan = dict(nkt_t=tuple(int(x) for x in nkt_t), NKT=NKT)
    return dis, srcs, ohs, plan


def _wrap_tiles(vec, p):
    """[NSH] -> [p, NTP//p] with vec[t*p+q] at (q, t); pad zeros."""
    v = np.zeros(NTP, np.float32)
    v[:NSH] = vec
    return np.ascontiguousarray(v.reshape(NTP // p, p).T)


def _pm(a, p):
    """[rows<=NTP, F] -> partition-major [p, (NTP//p)*F] (pad zeros)."""
    f = a.shape[1]
    v = np.zeros((NTP, f), np.float32)
    v[:a.shape[0]] = a
    return np.ascontiguousarray(
        v.reshape(NTP // p, p, f).transpose(1, 0, 2).reshape(p, -1))


def _unpm(a, p, f):
    """[p, (NTP//p)*F] -> [NSH, F]."""
    return np.ascontiguousarray(
        a.reshape(p, NTP // p, f).transpose(1, 0, 2).reshape(NTP, f)[:NSH])


def _stream_pm(table, stream_src, nkt):
    """Gather table rows [N, F] by stream -> [128, nkt*F] partition-major."""
    f = table.shape[1]
    g = table[stream_src]                     # [nkt*128, F]
    return np.ascontiguousarray(
        g.reshape(nkt, 128, f).transpose(1, 0, 2).reshape(128, nkt * f))


# --------------------------------------------------------------------------
# launch 1: h = relu(x@w1+b1); u0 = dis*h
# --------------------------------------------------------------------------

def _build_L1():
    nc = bacc.Bacc(None, target_bir_lowering=False, debug=False)
    xT = nc.dram_tensor("xT", [F_IN, NTP], mybir.dt.float8e4, kind="ExternalInput")
    w1 = nc.dram_tensor("w1", [F_IN, H], mybir.dt.float8e4, kind="ExternalInput")
    b1r = nc.dram_tensor("b1r", [1, H], mybir.dt.bfloat16, kind="ExternalInput")
    disw = nc.dram_tensor("disw", [128, NT], mybir.dt.float32, kind="ExternalInput")
    h_o = nc.dram_tensor("h", [128, NT * H], mybir.dt.bfloat16, kind="ExternalOutput")
    u0_o = nc.dram_tensor("u0", [128, NT * H], mybir.dt.bfloat16, kind="ExternalOutput")

    with tile.TileContext(nc) as tc:
        with (
            tc.tile_pool(name="per", bufs=1) as per,
            tc.tile_pool(name="sb", bufs=4) as sb,
            tc.tile_pool(name="ps", bufs=4, space="PSUM") as ps,
        ):
            xT0 = per.tile([128, NTP], mybir.dt.float8e4)
            xT1 = per.tile([128, NTP], mybir.dt.float8e4)
            w1a = per.tile([128, H], mybir.dt.float8e4)
            w1b = per.tile([128, H], mybir.dt.float8e4)
            b1t = per.tile([1, H], mybir.dt.bfloat16)
            ones = per.tile([1, 128], mybir.dt.bfloat16)
            dt = per.tile([128, NT], mybir.dt.float32)
            h_sb = per.tile([128, NT, H], mybir.dt.bfloat16)
            u0_sb = per.tile([128, NT, H], mybir.dt.bfloat16)
            nc.sync.dma_start(xT0[:], xT[0:128, :])
            nc.sync.dma_start(xT1[:], xT[128:256, :])
            nc.sync.dma_start(w1a[:], w1[0:128, :])
            nc.sync.dma_start(w1b[:], w1[128:256, :])
            nc.sync.dma_start(b1t[:], b1r[:])
            nc.sync.dma_start(dt[:], disw[:])
            nc.vector.memset(ones[:], 1.0)
            for t in range(NT):
                pt = ps.tile([128, H], mybir.dt.float32, tag="mm")
                cols = slice(t * 128, (t + 1) * 128)
                nc.tensor.matmul(pt[:], xT0[:, cols], w1a[:], start=True, stop=False)
                nc.tensor.matmul(pt[:], xT1[:, cols], w1b[:], start=False, stop=False)
                nc.tensor.matmul(pt[:], ones[:], b1t[:], start=False, stop=True)
                nc.scalar.activation(h_sb[:, t, :], pt[:], AF.Relu)
                nc.vector.tensor_scalar(u0_sb[:, t, :], pt[:], 0.0,
                                        dt[:, t:t + 1], ALU.max, ALU.mult)
            nc.sync.dma_start(h_o.rearrange("p (t f) -> p t f", f=H), h_sb[:])
            nc.sync.dma_start(u0_o.rearrange("p (t f) -> p t f", f=H), u0_sb[:])
    nc.compile()
    return nc


# --------------------------------------------------------------------------
# shared hop body: fp8 one-hot stream + fp8 value stream, psum[TW dst, H]
# --------------------------------------------------------------------------

def _hop_body(nc, sb, ps, plan, vst, ohst, seg_fn):
    nkt_t, NKT = plan["nkt_t"], plan["NKT"]
    vv = vst.rearrange("p (k f) -> p k f", f=H)
    ov = ohst.rearrange("p (k f) -> p k f", f=TW)
    blk = {}

    def get_blk(kt):
        b0 = (kt // VB) * VB
        if b0 not in blk:
            nb = min(VB, NKT - b0)
            vb = sb.tile([128, nb, H], mybir.dt.float8e4, tag="vb", bufs=3,
                         name=f"vb_{b0}")
            ob = sb.tile([128, nb, TW], mybir.dt.float8e4, tag="ob", bufs=3,
                         name=f"ob_{b0}")
            nc.sync.dma_start(vb[:], vv[:, b0:b0 + nb, :])
            nc.sync.dma_start(ob[:], ov[:, b0:b0 + nb, :])
            blk[b0] = (vb, ob)
        return blk[b0], kt - b0

    kt = 0
    for t in range(NT2):
        nkt = nkt_t[t]
        hp = ps.tile([TW, H], mybir.dt.float32, tag="hp", bufs=2,
                     name=f"hp_{t}")
        for i in range(nkt):
            (vb, ob), j = get_blk(kt)
            nc.tensor.matmul(hp[:], ob[:, j, :], vb[:, j, :],
                             start=(i == 0), stop=(i == nkt - 1))
            kt += 1
        seg_fn(t, hp)
    assert kt == NKT


# --------------------------------------------------------------------------
# launch 2: hop1 -> h1, u1
# --------------------------------------------------------------------------

def _build_L2(plan):
    NKT = plan["NKT"]
    nc = bacc.Bacc(None, target_bir_lowering=False, debug=False)
    vst = nc.dram_tensor("vst", [128, NKT * H], mybir.dt.float8e4, kind="ExternalInput")
    ohst = nc.dram_tensor("ohst", [128, NKT * TW], mybir.dt.float8e4, kind="ExternalInput")
    ow1 = nc.dram_tensor("ow1", [TW, NT2 * H], mybir.dt.bfloat16, kind="ExternalInput")
    ow2 = nc.dram_tensor("ow2", [TW, NT2 * H], mybir.dt.bfloat16, kind="ExternalInput")
    dtw = nc.dram_tensor("dtw", [TW, NT2], mybir.dt.float32, kind="ExternalInput")
    dt2w = nc.dram_tensor("dt2w", [TW, NT2], mybir.dt.float32, kind="ExternalInput")
    h1_o = nc.dram_tensor("h1", [TW, NT2 * H], mybir.dt.bfloat16, kind="ExternalOutput")
    u1_o = nc.dram_tensor("u1", [TW, NT2 * H], mybir.dt.bfloat16, kind="ExternalOutput")

    with tile.TileContext(nc) as tc:
        with (
            tc.tile_pool(name="per", bufs=1) as per,
            tc.tile_pool(name="sb", bufs=2) as sb,
            tc.tile_pool(name="ps", bufs=2, space="PSUM") as ps,
        ):
            ow1_t = per.tile([TW, NT2, H], mybir.dt.bfloat16)
            ow2_t = per.tile([TW, NT2, H], mybir.dt.bfloat16)
            dt = per.tile([TW, NT2], mybir.dt.float32)
            dt2 = per.tile([TW, NT2], mybir.dt.float32)
            h1_sb = per.tile([TW, NT2, H], mybir.dt.bfloat16)
            u1_sb = per.tile([TW, NT2, H], mybir.dt.bfloat16)
            nc.sync.dma_start(ow1_t[:], ow1.rearrange("p (t f) -> p t f", f=H))
            nc.sync.dma_start(ow2_t[:], ow2.rearrange("p (t f) -> p t f", f=H))
            nc.sync.dma_start(dt[:], dtw[:])
            nc.sync.dma_start(dt2[:], dt2w[:])

            def seg(t, hp):
                # h1 = dis*psum + ow1 ; u1 = dis^2*psum + ow2
                nc.vector.scalar_tensor_tensor(
                    h1_sb[:, t, :], hp[:], dt[:, t:t + 1], ow1_t[:, t, :],
                    ALU.mult, ALU.add)
                nc.vector.scalar_tensor_tensor(
                    u1_sb[:, t, :], hp[:], dt2[:, t:t + 1], ow2_t[:, t, :],
                    ALU.mult, ALU.add)

            _hop_body(nc, sb, ps, plan, vst, ohst, seg)
            nc.sync.dma_start(h1_o.rearrange("p (t f) -> p t f", f=H), h1_sb[:])
            nc.sync.dma_start(u1_o.rearrange("p (t f) -> p t f", f=H), u1_sb[:])
    nc.compile()
    return nc


# --------------------------------------------------------------------------
# launch 3: hop2 + dense MLP tail -> log_softmax logits
# --------------------------------------------------------------------------

def _build_L3(plan):
    NKT = plan["NKT"]
    nc = bacc.Bacc(None, target_bir_lowering=False, debug=False)
    vst = nc.dram_tensor("vst", [128, NKT * H], mybir.dt.float8e4, kind="ExternalInput")
    ohst = nc.dram_tensor("ohst", [128, NKT * TW], mybir.dt.float8e4, kind="ExternalInput")
    hT = nc.dram_tensor("hT", [H, NTP], mybir.dt.bfloat16, kind="ExternalInput")
    h1T = nc.dram_tensor("h1T", [H, NTP], mybir.dt.bfloat16, kind="ExternalInput")
    ow = nc.dram_tensor("ow", [TW, NT2 * H], mybir.dt.bfloat16, kind="ExternalInput")
    dtw = nc.dram_tensor("dtw", [TW, NT2], mybir.dt.float32, kind="ExternalInput")
    wp0 = nc.dram_tensor("wp0", [H, H], mybir.dt.bfloat16, kind="ExternalInput")
    wp1 = nc.dram_tensor("wp1", [H, H], mybir.dt.bfloat16, kind="ExternalInput")
    wp2 = nc.dram_tensor("wp2", [H, H], mybir.dt.bfloat16, kind="ExternalInput")
    bps = nc.dram_tensor("bps", [1, 3 * H], mybir.dt.bfloat16, kind="ExternalInput")
    w2d = nc.dram_tensor("w2", [3 * H, C], mybir.dt.bfloat16, kind="ExternalInput")
    b2d = nc.dram_tensor("b2", [1, C], mybir.dt.bfloat16, kind="ExternalInput")
    idt = nc.dram_tensor("idt", [C, C], mybir.dt.bfloat16, kind="ExternalInput")
    id64 = nc.dram_tensor("id64", [TW, TW], mybir.dt.bfloat16, kind="ExternalInput")
    lg_o = nc.dram_tensor("lg", [128, NT * C], mybir.dt.float32, kind="ExternalOutput")

    with tile.TileContext(nc) as tc:
        with (
            tc.tile_pool(name="per", bufs=1) as per,
            tc.tile_pool(name="sb", bufs=2) as sb,
            tc.tile_pool(name="ps", bufs=2, space="PSUM") as ps,
        ):
            ow_t = per.tile([TW, NT2, H], mybir.dt.bfloat16)
            dt = per.tile([TW, NT2], mybir.dt.float32)
            h2T_sb = per.tile([H, NTP], mybir.dt.bfloat16)
            wpt = [per.tile([H, H], mybir.dt.bfloat16, name=f"wpt{i}")
                   for i in range(3)]
            bps_t = per.tile([1, 3 * H], mybir.dt.bfloat16)
            w2t = [per.tile([H, C], mybir.dt.bfloat16, name=f"w2t{i}")
                   for i in range(3)]
            b2t = per.tile([1, C], mybir.dt.bfloat16)
            ones = per.tile([1, 512], mybir.dt.bfloat16)
            identC = per.tile([C, C], mybir.dt.bfloat16)
            ident64 = per.tile([TW, TW], mybir.dt.bfloat16)
            lg_sb = per.tile([128, NT, C], mybir.dt.float32)
            nc.sync.dma_start(ow_t[:], ow.rearrange("p (t f) -> p t f", f=H))
            nc.sync.dma_start(dt[:], dtw[:])
            for i, wd in enumerate((wp0, wp1, wp2)):
                nc.sync.dma_start(wpt[i][:], wd[:])
                nc.sync.dma_start(w2t[i][:], w2d[i * H:(i + 1) * H, :])
            nc.sync.dma_start(bps_t[:], bps[:])
            nc.sync.dma_start(b2t[:], b2d[:])
            nc.sync.dma_start(identC[:], idt[:])
            nc.sync.dma_start(ident64[:], id64[:])
            nc.vector.memset(ones[:], 1.0)

            def seg(t, hp):
                # h2 = dis*psum + ow ; transpose into h2T_sb column block
                cols = slice(t * TW, (t + 1) * TW)
                h2s = sb.tile([TW, H], mybir.dt.bfloat16, tag="h2s", bufs=3,
                              name=f"h2s_{t}")
                nc.vector.scalar_tensor_tensor(
                    h2s[:], hp[:], dt[:, t:t + 1], ow_t[:, t, :],
                    ALU.mult, ALU.add)
                tp = ps.tile([H, TW], mybir.dt.bfloat16, tag="tp", bufs=1,
                             name=f"tp_{t}")
                nc.tensor.transpose(tp[:], h2s[:], ident64[:])
                nc.vector.tensor_copy(h2T_sb[:, cols], tp[:])

            _hop_body(nc, sb, ps, plan, vst, ohst, seg)

            # dense MLP tail over the whole shard, TB 128-col blocks per step
            for tb0 in range(0, NT, TB):
                ntb = min(TB, NT - tb0)
                W = ntb * 128
                cols = slice(tb0 * 128, tb0 * 128 + W)
                ht_b = sb.tile([H, 512], mybir.dt.bfloat16, tag="htb", bufs=2,
                               name=f"htb_{tb0}")
                h1t_b = sb.tile([H, 512], mybir.dt.bfloat16, tag="h1tb", bufs=2,
                                name=f"h1tb_{tb0}")
                nc.sync.dma_start(ht_b[:, :W], hT[:, cols])
                nc.sync.dma_start(h1t_b[:, :W], h1T[:, cols])
                XTs = (ht_b[:, :W], h1t_b[:, :W], h2T_sb[:, cols])
                z = sb.tile([H, 3, 512], mybir.dt.bfloat16, tag="z", bufs=2,
                            name=f"z_{tb0}")
                for i in range(3):
                    yb = ps.tile([H, 512], mybir.dt.float32, tag="yb", bufs=2,
                                 name=f"yb_{tb0}_{i}")
                    nc.tensor.matmul(yb[:, :W], wpt[i][:], XTs[i],
                                     start=True, stop=False)
                    nc.tensor.matmul(yb[:, :W], bps_t[:, i * H:(i + 1) * H],
                                     ones[:, :W], start=False, stop=True)
                    nc.scalar.activation(z[:, i, :W], yb[:, :W], AF.Relu)
                lt = ps.tile([C, 512], mybir.dt.float32, tag="lt", bufs=2,
                             name=f"lt_{tb0}")
                for i in range(3):
                    nc.tensor.matmul(lt[:, :W], w2t[i][:], z[:, i, :W],
                                     start=(i == 0), stop=False)
                nc.tensor.matmul(lt[:, :W], b2t[:], ones[:, :W],
                                 start=False, stop=True)
                lts = sb.tile([C, 512], mybir.dt.bfloat16, tag="lts", bufs=2,
                              name=f"lts_{tb0}")
                nc.vector.tensor_copy(lts[:, :W], lt[:, :W])
                for j in range(ntb):
                    lgp = ps.tile([128, C], mybir.dt.bfloat16, tag="lgp", bufs=1,
                                  name=f"lgp_{tb0}_{j}")
                    nc.tensor.transpose(lgp[:], lts[:, j * 128:(j + 1) * 128],
                                        identC[:])
                    nc.vector.tensor_copy(lg_sb[:, tb0 + j, :], lgp[:])

            # batched log-softmax over [128, NT, C] (exp reuses lg_sb)
            negm = per.tile([128, NT, 1], mybir.dt.float32)
            xs = per.tile([128, NT, C], mybir.dt.float32)
            ss = per.tile([128, NT, 1], mybir.dt.float32)
            ls = per.tile([128, NT, 1], mybir.dt.float32)
            nc.vector.tensor_reduce(negm[:], lg_sb[:], mybir.AxisListType.X,
                                    ALU.max, negate=True)
            nc.vector.tensor_tensor(
                xs[:], lg_sb[:],
                negm[:].to_broadcast([128, NT, C]), ALU.add)
            nc.scalar.activation(lg_sb[:], xs[:], AF.Exp)
            nc.vector.tensor_reduce(ss[:], lg_sb[:], mybir.AxisListType.X,
                                    ALU.add)
            nc.scalar.activation(ls[:], ss[:], AF.Ln)
            nc.vector.tensor_tensor(
                lg_sb[:], xs[:],
                ls[:].to_broadcast([128, NT, C]), ALU.subtract)
            nc.sync.dma_start(lg_o.rearrange("p (t f) -> p t f", f=C), lg_sb[:])
    nc.compile()
    return nc


# --------------------------------------------------------------------------
# top-level entry
# --------------------------------------------------------------------------

def kernel(**inputs):
    x = np.asarray(inputs["x"], np.float32)
    edge_index = np.asarray(inputs["edge_index"])
    w1 = np.asarray(inputs["w1"], np.float32)
    b1 = np.asarray(inputs["b1"], np.float32)
    wps = [np.asarray(inputs[f"wp{i}"], np.float32) for i in range(3)]
    bps = [np.asarray(inputs[f"bp{i}"], np.float32) for i in range(3)]
    w2 = np.asarray(inputs["w2"], np.float32)
    b2 = np.asarray(inputs["b2"], np.float32)

    dis, srcs, ohs, plan = _prep_graph(edge_index)
    key = ("prog", plan["nkt_t"])
    if key not in _cache:
        _cache[key] = (_build_L1(), _build_L2(plan), _build_L3(plan))
    ncL1, ncL2, ncL3 = _cache[key]
    NKT = plan["NKT"]

    disw_c = [_wrap_tiles(dis[c * NSH:(c + 1) * NSH], 128)
              for c in range(NCORE)]
    dis64_c = [_wrap_tiles(dis[c * NSH:(c + 1) * NSH], TW)
               for c in range(NCORE)]
    dis64sq_c = [_wrap_tiles(dis[c * NSH:(c + 1) * NSH] ** 2, TW)
                 for c in range(NCORE)]

    # ---- L1
    in1 = []
    for c in range(NCORE):
        xT = np.zeros((F_IN, NTP), FP8)
        xT[:, :NSH] = x[c * NSH:(c + 1) * NSH].T.astype(FP8)
        in1.append({"xT": xT, "w1": w1.astype(FP8),
                    "b1r": b1[None, :].astype(BF16), "disw": disw_c[c]})
    _last_runs.clear()
    _last_runs.append(("L1", ncL1, in1))
    r1 = run_bass_kernel_spmd(ncL1, in1, list(range(NCORE)))
    h_c = [_unpm(r1.results[c]["h"], 128, H).astype(np.float32)
           for c in range(NCORE)]
    u0f = np.concatenate([_unpm(r1.results[c]["u0"], 128, H)
                          for c in range(NCORE)]).astype(np.float32)

    # ---- L2 (host materializes the u0[src] stream per core)
    in2 = []
    u0f8 = u0f.astype(FP8)
    for c in range(NCORE):
        dsh = dis[c * NSH:(c + 1) * NSH]
        u0own = u0f[c * NSH:(c + 1) * NSH]
        in2.append({
            "vst": _stream_pm(u0f8, srcs[c], NKT), "ohst": ohs[c],
            "ow1": _pm(dsh[:, None] * u0own, TW).astype(BF16),
            "ow2": _pm((dsh ** 2)[:, None] * u0own, TW).astype(BF16),
            "dtw": dis64_c[c], "dt2w": dis64sq_c[c],
        })
    _last_runs.append(("L2", ncL2, in2))
    r2 = run_bass_kernel_spmd(ncL2, in2, list(range(NCORE)))
    h1_c = [_unpm(r2.results[c]["h1"], TW, H).astype(np.float32)
            for c in range(NCORE)]
    u1f = np.concatenate([_unpm(r2.results[c]["u1"], TW, H)
                          for c in range(NCORE)]).astype(np.float32)

    # ---- L3
    def padT(a):
        out = np.zeros((H, NTP), BF16)
        out[:, :a.shape[0]] = a.T.astype(BF16)
        return out

    bps_cat = np.concatenate(bps)[None, :].astype(BF16)
    u1f8 = u1f.astype(FP8)
    in3 = []
    for c in range(NCORE):
        dsh = dis[c * NSH:(c + 1) * NSH]
        u1own = u1f[c * NSH:(c + 1) * NSH]
        in3.append({
            "vst": _stream_pm(u1f8, srcs[c], NKT), "ohst": ohs[c],
            "hT": padT(h_c[c]), "h1T": padT(h1_c[c]),
            "ow": _pm(dsh[:, None] * u1own, TW).astype(BF16),
            "dtw": dis64_c[c],
            "wp0": wps[0].astype(BF16), "wp1": wps[1].astype(BF16),
            "wp2": wps[2].astype(BF16), "bps": bps_cat,
            "w2": w2.astype(BF16), "b2": b2[None, :].astype(BF16),
            "idt": np.eye(C, dtype=BF16),
            "id64": np.eye(TW, dtype=BF16),
        })
    _last_runs.append(("L3", ncL3, in3))
    r3 = run_bass_kernel_spmd(ncL3, in3, list(range(NCORE)))
    out = np.concatenate([_unpm(r3.results[c]["lg"], 128, C)
                          for c in range(NCORE)])
    return out.astype(np.float32)


# revision 12
# speedup vs baseline: 1.1028x; 1.1028x over previous
"""MixHop GNN (2-hop GCN propagation + MLP head) on 8 Trainium2 NeuronCores.

Strategy (node-sharded by dst, streaming — no on-device gather):
  norm factorization:  norm = dis[src]*dis[dst] ->  hop(v) = dis * S(dis * v)
  with S the plain scatter-sum over edges; self loops handled as a direct
  per-node add in the tail (never materialized as edges).

  Between launches the HOST materializes the per-edge value stream
  v_e = u[src_e] (fp8) in dst-grouped k-tile order, plus a STATIC fp8
  one-hot stream for the scatter matrices (built once, reused by both
  hops; dst tiles are 64 wide to halve the one-hot bytes).  Each core
  consumes both streams SEQUENTIALLY with fat DMA descriptors; the
  scatter-sum runs on the Tensor engine as
      psum[dst_tile 64, H] += OneHotT(fp8)^T @ v_tile(fp8)
  PSUM accumulation is in-order => no scatter races.  No SWDGE descriptor
  generation, no random-access DMA, no on-chip one-hot build.

  3 SPMD launches over 8 cores:
    L1: h = relu(x@w1+b1), u0 = dis*h                (row shard per core)
    L2: hop1 over u0-stream -> h1, u1 shards
    L3: hop2 over u1-stream + dense MLP tail -> log_softmax logits shard
"""

import numpy as np
import ml_dtypes

import concourse.bacc as bacc
import concourse.bass as bass
import concourse.tile as tile
from concourse import mybir
from concourse.bass_utils import run_bass_kernel_spmd

BF16 = ml_dtypes.bfloat16
FP8 = ml_dtypes.float8_e4m3
AF = mybir.ActivationFunctionType
ALU = mybir.AluOpType

N, E, F_IN, H, C = 100000, 1600000, 256, 64, 40
NCORE = 8
NSH = N // NCORE            # 12500 nodes per core
NT = (NSH + 127) // 128     # 98 128-row tiles per core (L1 + logits layout)
NTP = NT * 128              # 12544 padded rows
TW = 128                    # dst tile width for the hop scatter
NT2 = NTP // TW             # 196 hop dst tiles per core
VB = 96                     # k-tiles per stream staging block
TB = 4                      # 128-col blocks per dense-tail step in L3

_cache = {}
_last_runs = []


# --------------------------------------------------------------------------
# host-side graph partitioning / padding plan
# --------------------------------------------------------------------------

def _prep_graph(edge_index):
    src = np.asarray(edge_index[0], dtype=np.int64)
    dst = np.asarray(edge_index[1], dtype=np.int64)
    deg = (np.bincount(dst, minlength=N) + 1).astype(np.float32)  # + self loop
    dis = (1.0 / np.sqrt(deg)).astype(np.float32)

    per_core = []
    cnts = np.zeros((NCORE, NT2), np.int64)
    for c in range(NCORE):
        sel = (dst // NSH) == c
        s_g = src[sel]
        d_l = (dst[sel] - c * NSH).astype(np.int64)
        t_id = d_l // TW
        order = np.argsort(t_id, kind="stable")
        s_g, d_l, t_id = s_g[order], d_l[order], t_id[order]
        cnts[c] = np.bincount(t_id, minlength=NT2)
        per_core.append((s_g, d_l, t_id))

    nkt_t = np.ceil(cnts.max(axis=0) / 128.0).astype(np.int64)  # k-tiles/seg
    nkt_t = np.maximum(nkt_t, 1)
    off_t = np.zeros(NT2 + 1, np.int64)
    np.cumsum(nkt_t, out=off_t[1:])
    NKT = int(off_t[-1])

    srcs, ohs = [], []
    for c in range(NCORE):
        s_g, d_l, t_id = per_core[c]
        start = np.zeros(NT2, np.int64)
        np.cumsum(cnts[c], out=start)
        start = np.concatenate([[0], start[:-1]])
        rank = np.arange(len(t_id)) - start[t_id]
        pos = off_t[t_id] * 128 + rank
        stream_src = np.zeros(NKT * 128, np.int64)
        stream_src[pos] = s_g
        srcs.append(stream_src)
        # static fp8 one-hot stream: row e has 1 at dst_local_in_tile
        oh = np.zeros((NKT * 128, TW), FP8)
        oh[pos, d_l - TW * t_id] = 1
        ohs.append(np.ascontiguousarray(
            oh.reshape(NKT, 128, TW).transpose(1, 0, 2)
            .reshape(128, NKT * TW)))

    plan = dict(nkt_t=tuple(int(x) for x in nkt_t), NKT=NKT)
    return dis, srcs, ohs, plan


def _wrap_tiles(vec, p):
    """[NSH] -> [p, NTP//p] with vec[t*p+q] at (q, t); pad zeros."""
    v = np.zeros(NTP, np.float32)
    v[:NSH] = vec
    return np.ascontiguousarray(v.reshape(NTP // p, p).T)


def _pm(a, p):
    """[rows<=NTP, F] -> partition-major [p, (NTP//p)*F] (pad zeros)."""
    f = a.shape[1]
    v = np.zeros((NTP, f), np.float32)
    v[:a.shape[0]] = a
    return np.ascontiguousarray(
        v.reshape(NTP // p, p, f).transpose(1, 0, 2).reshape(p, -1))


def _unpm(a, p, f):
    """[p, (NTP//p)*F] -> [NSH, F]."""
    return np.ascontiguousarray(
        a.reshape(p, NTP // p, f).transpose(1, 0, 2).reshape(NTP, f)[:NSH])


def _stream_pm(table, stream_src, nkt):
    """Gather table rows [N, F] by stream -> [128, nkt*F] partition-major."""
    f = table.shape[1]
    g = table[stream_src]                     # [nkt*128, F]
    return np.ascontiguousarray(
        g.reshape(nkt, 128, f).transpose(1, 0, 2).reshape(128, nkt * f))


# --------------------------------------------------------------------------
# launch 1: h = relu(x@w1+b1); u0 = dis*h
# --------------------------------------------------------------------------

def _build_L1():
    nc = bacc.Bacc(None, target_bir_lowering=False, debug=False)
    xT = nc.dram_tensor("xT", [F_IN, NTP], mybir.dt.float8e4, kind="ExternalInput")
    w1 = nc.dram_tensor("w1", [F_IN, H], mybir.dt.float8e4, kind="ExternalInput")
    b1r = nc.dram_tensor("b1r", [1, H], mybir.dt.bfloat16, kind="ExternalInput")
    disw = nc.dram_tensor("disw", [128, NT], mybir.dt.float32, kind="ExternalInput")
    h_o = nc.dram_tensor("h", [128, NT * H], mybir.dt.bfloat16, kind="ExternalOutput")
    u0_o = nc.dram_tensor("u0", [128, NT * H], mybir.dt.bfloat16, kind="ExternalOutput")

    with tile.TileContext(nc) as tc:
        with (
            tc.tile_pool(name="per", bufs=1) as per,
            tc.tile_pool(name="sb", bufs=4) as sb,
            tc.tile_pool(name="ps", bufs=4, space="PSUM") as ps,
        ):
            xT0 = per.tile([128, NTP], mybir.dt.float8e4)
            xT1 = per.tile([128, NTP], mybir.dt.float8e4)
            w1a = per.tile([128, H], mybir.dt.float8e4)
            w1b = per.tile([128, H], mybir.dt.float8e4)
            b1t = per.tile([1, H], mybir.dt.bfloat16)
            ones = per.tile([1, 128], mybir.dt.bfloat16)
            dt = per.tile([128, NT], mybir.dt.float32)
            h_sb = per.tile([128, NT, H], mybir.dt.bfloat16)
            u0_sb = per.tile([128, NT, H], mybir.dt.bfloat16)
            nc.sync.dma_start(xT0[:], xT[0:128, :])
            nc.sync.dma_start(xT1[:], xT[128:256, :])
            nc.sync.dma_start(w1a[:], w1[0:128, :])
            nc.sync.dma_start(w1b[:], w1[128:256, :])
            nc.sync.dma_start(b1t[:], b1r[:])
            nc.sync.dma_start(dt[:], disw[:])
            nc.vector.memset(ones[:], 1.0)
            for t in range(NT):
                pt = ps.tile([128, H], mybir.dt.float32, tag="mm")
                cols = slice(t * 128, (t + 1) * 128)
                nc.tensor.matmul(pt[:], xT0[:, cols], w1a[:], start=True, stop=False)
                nc.tensor.matmul(pt[:], xT1[:, cols], w1b[:], start=False, stop=False)
                nc.tensor.matmul(pt[:], ones[:], b1t[:], start=False, stop=True)
                nc.scalar.activation(h_sb[:, t, :], pt[:], AF.Relu)
                nc.vector.tensor_scalar(u0_sb[:, t, :], pt[:], 0.0,
                                        dt[:, t:t + 1], ALU.max, ALU.mult)
            nc.sync.dma_start(h_o.rearrange("p (t f) -> p t f", f=H), h_sb[:])
            nc.sync.dma_start(u0_o.rearrange("p (t f) -> p t f", f=H), u0_sb[:])
    nc.compile()
    return nc


# --------------------------------------------------------------------------
# shared hop body: fp8 one-hot stream + fp8 value stream, psum[TW dst, H]
# --------------------------------------------------------------------------

def _hop_body(nc, sb, ps, plan, vst, ohst, seg_fn):
    nkt_t, NKT = plan["nkt_t"], plan["NKT"]
    vv = vst.rearrange("p (k f) -> p k f", f=H)
    ov = ohst.rearrange("p (k f) -> p k f", f=TW)
    blk = {}

    def get_blk(kt):
        b0 = (kt // VB) * VB
        if b0 not in blk:
            nb = min(VB, NKT - b0)
            vb = sb.tile([128, nb, H], mybir.dt.float8e4, tag="vb", bufs=3,
                         name=f"vb_{b0}")
            ob = sb.tile([128, nb, TW], mybir.dt.float8e4, tag="ob", bufs=3,
                         name=f"ob_{b0}")
            nc.sync.dma_start(vb[:], vv[:, b0:b0 + nb, :])
            nc.sync.dma_start(ob[:], ov[:, b0:b0 + nb, :])
            blk[b0] = (vb, ob)
        return blk[b0], kt - b0

    kt = 0
    for t in range(NT2):
        nkt = nkt_t[t]
        hp = ps.tile([TW, H], mybir.dt.float32, tag="hp", bufs=2,
                     name=f"hp_{t}")
        for i in range(nkt):
            (vb, ob), j = get_blk(kt)
            nc.tensor.matmul(hp[:], ob[:, j, :], vb[:, j, :],
                             start=(i == 0), stop=(i == nkt - 1))
            kt += 1
        seg_fn(t, hp)
    assert kt == NKT


# --------------------------------------------------------------------------
# launch 2: hop1 -> h1, u1
# --------------------------------------------------------------------------

def _build_L2(plan):
    NKT = plan["NKT"]
    nc = bacc.Bacc(None, target_bir_lowering=False, debug=False)
    vst = nc.dram_tensor("vst", [128, NKT * H], mybir.dt.float8e4, kind="ExternalInput")
    ohst = nc.dram_tensor("ohst", [128, NKT * TW], mybir.dt.float8e4, kind="ExternalInput")
    ow1 = nc.dram_tensor("ow1", [TW, NT2 * H], mybir.dt.bfloat16, kind="ExternalInput")
    dtw = nc.dram_tensor("dtw", [TW, NT2], mybir.dt.float32, kind="ExternalInput")
    h1_o = nc.dram_tensor("h1", [TW, NT2 * H], mybir.dt.bfloat16, kind="ExternalOutput")
    u1_o = nc.dram_tensor("u1", [TW, NT2 * H], mybir.dt.bfloat16, kind="ExternalOutput")

    with tile.TileContext(nc) as tc:
        with (
            tc.tile_pool(name="per", bufs=1) as per,
            tc.tile_pool(name="sb", bufs=2) as sb,
            tc.tile_pool(name="ps", bufs=2, space="PSUM") as ps,
        ):
            ow1_t = per.tile([TW, NT2, H], mybir.dt.bfloat16)
            dt = per.tile([TW, NT2], mybir.dt.float32)
            h1_sb = per.tile([TW, NT2, H], mybir.dt.bfloat16)
            u1_sb = per.tile([TW, NT2, H], mybir.dt.bfloat16)
            nc.sync.dma_start(ow1_t[:], ow1.rearrange("p (t f) -> p t f", f=H))
            nc.sync.dma_start(dt[:], dtw[:])

            def seg(t, hp):
                # h1 = dis*psum + ow1 ; u1 = dis*h1
                nc.vector.scalar_tensor_tensor(
                    h1_sb[:, t, :], hp[:], dt[:, t:t + 1], ow1_t[:, t, :],
                    ALU.mult, ALU.add)
                nc.vector.tensor_scalar(
                    u1_sb[:, t, :], h1_sb[:, t, :], dt[:, t:t + 1], None,
                    ALU.mult)

            _hop_body(nc, sb, ps, plan, vst, ohst, seg)
            nc.sync.dma_start(h1_o.rearrange("p (t f) -> p t f", f=H), h1_sb[:])
            nc.sync.dma_start(u1_o.rearrange("p (t f) -> p t f", f=H), u1_sb[:])
    nc.compile()
    return nc


# --------------------------------------------------------------------------
# launch 3: hop2 + dense MLP tail -> log_softmax logits
# --------------------------------------------------------------------------

def _build_L3(plan):
    NKT = plan["NKT"]
    nc = bacc.Bacc(None, target_bir_lowering=False, debug=False)
    vst = nc.dram_tensor("vst", [128, NKT * H], mybir.dt.float8e4, kind="ExternalInput")
    ohst = nc.dram_tensor("ohst", [128, NKT * TW], mybir.dt.float8e4, kind="ExternalInput")
    hT = nc.dram_tensor("hT", [H, NTP], mybir.dt.bfloat16, kind="ExternalInput")
    h1T = nc.dram_tensor("h1T", [H, NTP], mybir.dt.bfloat16, kind="ExternalInput")
    ow = nc.dram_tensor("ow", [TW, NT2 * H], mybir.dt.bfloat16, kind="ExternalInput")
    dtw = nc.dram_tensor("dtw", [TW, NT2], mybir.dt.float32, kind="ExternalInput")
    wp0 = nc.dram_tensor("wp0", [H, H], mybir.dt.bfloat16, kind="ExternalInput")
    wp1 = nc.dram_tensor("wp1", [H, H], mybir.dt.bfloat16, kind="ExternalInput")
    wp2 = nc.dram_tensor("wp2", [H, H], mybir.dt.bfloat16, kind="ExternalInput")
    bps = nc.dram_tensor("bps", [1, 3 * H], mybir.dt.bfloat16, kind="ExternalInput")
    w2d = nc.dram_tensor("w2", [3 * H, C], mybir.dt.bfloat16, kind="ExternalInput")
    b2d = nc.dram_tensor("b2", [1, C], mybir.dt.bfloat16, kind="ExternalInput")
    idt = nc.dram_tensor("idt", [C, C], mybir.dt.bfloat16, kind="ExternalInput")
    id64 = nc.dram_tensor("id64", [TW, TW], mybir.dt.bfloat16, kind="ExternalInput")
    lg_o = nc.dram_tensor("lg", [128, NT * C], mybir.dt.float32, kind="ExternalOutput")

    with tile.TileContext(nc) as tc:
        with (
            tc.tile_pool(name="per", bufs=1) as per,
            tc.tile_pool(name="sb", bufs=2) as sb,
            tc.tile_pool(name="ps", bufs=2, space="PSUM") as ps,
        ):
            ow_t = per.tile([TW, NT2, H], mybir.dt.bfloat16)
            dt = per.tile([TW, NT2], mybir.dt.float32)
            h2T_sb = per.tile([H, NTP], mybir.dt.bfloat16)
            wpt = [per.tile([H, H], mybir.dt.bfloat16, name=f"wpt{i}")
                   for i in range(3)]
            bps_t = per.tile([1, 3 * H], mybir.dt.bfloat16)
            w2t = [per.tile([H, C], mybir.dt.bfloat16, name=f"w2t{i}")
                   for i in range(3)]
            b2t = per.tile([1, C], mybir.dt.bfloat16)
            ones = per.tile([1, 512], mybir.dt.bfloat16)
            identC = per.tile([C, C], mybir.dt.bfloat16)
            ident64 = per.tile([TW, TW], mybir.dt.bfloat16)
            lg_sb = per.tile([128, NT, C], mybir.dt.float32)
            nc.sync.dma_start(ow_t[:], ow.rearrange("p (t f) -> p t f", f=H))
            nc.sync.dma_start(dt[:], dtw[:])
            for i, wd in enumerate((wp0, wp1, wp2)):
                nc.sync.dma_start(wpt[i][:], wd[:])
                nc.sync.dma_start(w2t[i][:], w2d[i * H:(i + 1) * H, :])
            nc.sync.dma_start(bps_t[:], bps[:])
            nc.sync.dma_start(b2t[:], b2d[:])
            nc.sync.dma_start(identC[:], idt[:])
            nc.sync.dma_start(ident64[:], id64[:])
            nc.vector.memset(ones[:], 1.0)

            def seg(t, hp):
                # h2 = dis*psum + ow ; transpose into h2T_sb column block
                cols = slice(t * TW, (t + 1) * TW)
                h2s = sb.tile([TW, H], mybir.dt.bfloat16, tag="h2s", bufs=3,
                              name=f"h2s_{t}")
                nc.vector.scalar_tensor_tensor(
                    h2s[:], hp[:], dt[:, t:t + 1], ow_t[:, t, :],
                    ALU.mult, ALU.add)
                tp = ps.tile([H, TW], mybir.dt.bfloat16, tag="tp", bufs=1,
                             name=f"tp_{t}")
                nc.tensor.transpose(tp[:], h2s[:], ident64[:])
                nc.vector.tensor_copy(h2T_sb[:, cols], tp[:])

            _hop_body(nc, sb, ps, plan, vst, ohst, seg)

            # dense MLP tail over the whole shard, TB 128-col blocks per step
            for tb0 in range(0, NT, TB):
                ntb = min(TB, NT - tb0)
                W = ntb * 128
                cols = slice(tb0 * 128, tb0 * 128 + W)
                ht_b = sb.tile([H, 512], mybir.dt.bfloat16, tag="htb", bufs=2,
                               name=f"htb_{tb0}")
                h1t_b = sb.tile([H, 512], mybir.dt.bfloat16, tag="h1tb", bufs=2,
                                name=f"h1tb_{tb0}")
                nc.sync.dma_start(ht_b[:, :W], hT[:, cols])
                nc.sync.dma_start(h1t_b[:, :W], h1T[:, cols])
                XTs = (ht_b[:, :W], h1t_b[:, :W], h2T_sb[:, cols])
                z = sb.tile([H, 3, 512], mybir.dt.bfloat16, tag="z", bufs=2,
                            name=f"z_{tb0}")
                for i in range(3):
                    yb = ps.tile([H, 512], mybir.dt.float32, tag="yb", bufs=2,
                                 name=f"yb_{tb0}_{i}")
                    nc.tensor.matmul(yb[:, :W], wpt[i][:], XTs[i],
                                     start=True, stop=False)
                    nc.tensor.matmul(yb[:, :W], bps_t[:, i * H:(i + 1) * H],
                                     ones[:, :W], start=False, stop=True)
                    nc.scalar.activation(z[:, i, :W], yb[:, :W], AF.Relu)
                lt = ps.tile([C, 512], mybir.dt.float32, tag="lt", bufs=2,
                             name=f"lt_{tb0}")
                for i in range(3):
                    nc.tensor.matmul(lt[:, :W], w2t[i][:], z[:, i, :W],
                                     start=(i == 0), stop=False)
                nc.tensor.matmul(lt[:, :W], b2t[:], ones[:, :W],
                                 start=False, stop=True)
                lts = sb.tile([C, 512], mybir.dt.bfloat16, tag="lts", bufs=2,
                              name=f"lts_{tb0}")
                nc.vector.tensor_copy(lts[:, :W], lt[:, :W])
                for j in range(ntb):
                    lgp = ps.tile([128, C], mybir.dt.bfloat16, tag="lgp", bufs=1,
                                  name=f"lgp_{tb0}_{j}")
                    nc.tensor.transpose(lgp[:], lts[:, j * 128:(j + 1) * 128],
                                        identC[:])
                    nc.vector.tensor_copy(lg_sb[:, tb0 + j, :], lgp[:])

            # batched log-softmax over [128, NT, C] (exp reuses lg_sb)
            negm = per.tile([128, NT, 1], mybir.dt.float32)
            xs = per.tile([128, NT, C], mybir.dt.float32)
            ss = per.tile([128, NT, 1], mybir.dt.float32)
            ls = per.tile([128, NT, 1], mybir.dt.float32)
            nc.vector.tensor_reduce(negm[:], lg_sb[:], mybir.AxisListType.X,
                                    ALU.max, negate=True)
            nc.vector.tensor_tensor(
                xs[:], lg_sb[:],
                negm[:].to_broadcast([128, NT, C]), ALU.add)
            nc.scalar.activation(lg_sb[:], xs[:], AF.Exp)
            nc.vector.tensor_reduce(ss[:], lg_sb[:], mybir.AxisListType.X,
                                    ALU.add)
            nc.scalar.activation(ls[:], ss[:], AF.Ln)
            nc.vector.tensor_tensor(
                lg_sb[:], xs[:],
                ls[:].to_broadcast([128, NT, C]), ALU.subtract)
            nc.sync.dma_start(lg_o.rearrange("p (t f) -> p t f", f=C), lg_sb[:])
    nc.compile()
    return nc


# --------------------------------------------------------------------------
# top-level entry
# --------------------------------------------------------------------------

def kernel(**inputs):
    x = np.asarray(inputs["x"], np.float32)
    edge_index = np.asarray(inputs["edge_index"])
    w1 = np.asarray(inputs["w1"], np.float32)
    b1 = np.asarray(inputs["b1"], np.float32)
    wps = [np.asarray(inputs[f"wp{i}"], np.float32) for i in range(3)]
    bps = [np.asarray(inputs[f"bp{i}"], np.float32) for i in range(3)]
    w2 = np.asarray(inputs["w2"], np.float32)
    b2 = np.asarray(inputs["b2"], np.float32)

    dis, srcs, ohs, plan = _prep_graph(edge_index)
    key = ("prog", plan["nkt_t"])
    if key not in _cache:
        _cache[key] = (_build_L1(), _build_L2(plan), _build_L3(plan))
    ncL1, ncL2, ncL3 = _cache[key]
    NKT = plan["NKT"]

    disw_c = [_wrap_tiles(dis[c * NSH:(c + 1) * NSH], 128)
              for c in range(NCORE)]
    dis64_c = [_wrap_tiles(dis[c * NSH:(c + 1) * NSH], TW)
               for c in range(NCORE)]

    # ---- L1
    in1 = []
    for c in range(NCORE):
        xT = np.zeros((F_IN, NTP), FP8)
        xT[:, :NSH] = x[c * NSH:(c + 1) * NSH].T.astype(FP8)
        in1.append({"xT": xT, "w1": w1.astype(FP8),
                    "b1r": b1[None, :].astype(BF16), "disw": disw_c[c]})
    _last_runs.clear()
    _last_runs.append(("L1", ncL1, in1))
    r1 = run_bass_kernel_spmd(ncL1, in1, list(range(NCORE)))
    h_c = [_unpm(r1.results[c]["h"], 128, H).astype(np.float32)
           for c in range(NCORE)]
    u0f = np.concatenate([_unpm(r1.results[c]["u0"], 128, H)
                          for c in range(NCORE)]).astype(np.float32)

    # ---- L2 (host materializes the u0[src] stream per core)
    in2 = []
    u0f8 = u0f.astype(FP8)
    for c in range(NCORE):
        dsh = dis[c * NSH:(c + 1) * NSH]
        u0own = u0f[c * NSH:(c + 1) * NSH]
        in2.append({
            "vst": _stream_pm(u0f8, srcs[c], NKT), "ohst": ohs[c],
            "ow1": _pm(dsh[:, None] * u0own, TW).astype(BF16),
            "dtw": dis64_c[c],
        })
    _last_runs.append(("L2", ncL2, in2))
    r2 = run_bass_kernel_spmd(ncL2, in2, list(range(NCORE)))
    h1_c = [_unpm(r2.results[c]["h1"], TW, H).astype(np.float32)
            for c in range(NCORE)]
    u1f = np.concatenate([_unpm(r2.results[c]["u1"], TW, H)
                          for c in range(NCORE)]).astype(np.float32)

    # ---- L3
    def padT(a):
        out = np.zeros((H, NTP), BF16)
        out[:, :a.shape[0]] = a.T.astype(BF16)
        return out

    bps_cat = np.concatenate(bps)[None, :].astype(BF16)
    u1f8 = u1f.astype(FP8)
    in3 = []
    for c in range(NCORE):
        dsh = dis[c * NSH:(c + 1) * NSH]
        u1own = u1f[c * NSH:(c + 1) * NSH]
        in3.append({
            "vst": _stream_pm(u1f8, srcs[c], NKT), "ohst": ohs[c],
            "hT": padT(h_c[c]), "h1T": padT(h1_c[c]),
            "ow": _pm(dsh[:, None] * u1own, TW).astype(BF16),
            "dtw": dis64_c[c],
            "wp0": wps[0].astype(BF16), "wp1": wps[1].astype(BF16),
            "wp2": wps[2].astype(BF16), "bps": bps_cat,
            "w2": w2.astype(BF16), "b2": b2[None, :].astype(BF16),
            "idt": np.eye(C, dtype=BF16),
            "id64": np.eye(TW, dtype=BF16),
        })
    _last_runs.append(("L3", ncL3, in3))
    r3 = run_bass_kernel_spmd(ncL3, in3, list(range(NCORE)))
    out = np.concatenate([_unpm(r3.results[c]["lg"], 128, C)
                          for c in range(NCORE)])
    return out.astype(np.float32)
